# revision 68
# baseline (speedup 1.0000x reference)
"""Trainium2 Bass kernel for nn_ConvBlock (SepGconv + LayerNorm + GELU MLP).

Computes, for full inputs:
    a   = einsum('bsc,brsd,dc->brc', x, kernel_basis, kernel_W) + conv_bias
    a   = LayerNorm(a) * ln_scale + ln_bias          (over channels, eps=1e-6)
    out = gelu_tanh(a @ W1 + b1) @ W2 + b2

Shapes: B=2, N=1024 (R=S=N), H=64, D=32, WF=4.

Sharding: the (B*R)=2048 output rows split into 8 contiguous shards of 256
rows, one per NeuronCore. Each core reads its kernel_basis shard once
(memory-bound), contracts over all S on-chip, and runs the LN/MLP tail
locally. x / weights are replicated.

Perf strategy (HBM-bound at ~360 GB/s per core; ~58.5 us vs the 135 us
session baseline):
- kernel_basis streams in mixed precision: s-chunks 0..K8-1 as fp8 e3m4
  (1 B/elem; values pre-scaled by KS, with 1/KS folded into those chunks'
  x tiles) and the rest bf16 (2 B/elem) -- ~12.6 MB/core. The gate is
  rel_err < 2e-2; this measures 1.10e-2, deterministic (fixed inputs,
  fixed quantization, f32 PSUM accumulation). All tiles are prefetched
  into SBUF up front (SBUF is ~26 MB) so the DMA queues never stall on
  buffer recycling; the last j-block arrives in 3 pieces so PE can chase
  it (finer splits trickle: sub-2KB descriptors run below bus rate).
- dma_start issue costs ~0.6 us of serial sequencer time each, so the kb
  stream is issued alone on the sync queue while x + all small constants
  ride in packed-blob DMAs issued from the (otherwise idle) Scalar
  engine's queue. MLP/LN weights are pre-folded and packed bf16 (fp32
  matmuls cost 2 HW passes, so the whole tail runs bf16).
- Each matmul is psum[c, (r,d)] += x[s,c]^T @ kb[s,(r,d)] with N=512
  (16 rows x 32 d), K=128 s-chunk; the d-reduction against kernel_W runs
  on DVE (multiply by W broadcast + free-axis reduce). PE warm-up runs on
  a memset tile so the ~3 us HAM ramp to 2.4 GHz hides under x's DMA
  latency (dropping it makes j0-j2 run at mid p-state: measured worse).
- LayerNorm stats are accumulated DURING the stream: per j-block the
  channel sums of a+cb and its square land in a persistent PSUM tile via
  a per-quarter ones-matmul (squares on the Scalar engine -- its gelu
  table also holds Square). Row quarters then only need a short
  rsqrt(quake seed + Newton)/scale/MLP chain, staggered through the
  j-loop so every in-order engine-queue entry's inputs are long-ready;
  ln_scale/ln_bias are folded into W1/b1 on the host. Only the last
  quarter's chain (~6 us) runs after the stream.
"""

import os

import numpy as np

import concourse.bass as bass
import concourse.tile as tile
from concourse import mybir
from concourse.bass_utils import run_bass_kernel_spmd


def _ensure_axon_hooks():
    """bass_utils imports antenv.axon_hooks when trace=True under axon; some
    images ship antenv without that module. Register a functional stand-in
    (driving NTFF capture via libaxon_pjrt.so) so tracing works, degrading
    to hook=None (no trace, run still works) if the .so is unavailable."""
    import sys
    import types

    try:
        import antenv.axon_hooks  # noqa: F401

        return
    except ImportError:
        pass
    try:
        import antenv
    except ImportError:
        antenv = types.ModuleType("antenv")
        sys.modules["antenv"] = antenv

    mod = types.ModuleType("antenv.axon_hooks")
    mod._hook = None

    def set_axon_ntff_profile_hook(h):
        mod._hook = h

    def get_axon_ntff_profile_hook():
        if mod._hook is None:
            try:
                from trn_agent_boot.trn_boot import _ntff_profile_via_ctypes

                so_path = "/opt/axon/libaxon_pjrt.so"
                if os.path.exists(so_path):
                    mod._hook = _ntff_profile_via_ctypes(so_path)
            except Exception:
                mod._hook = None
        return mod._hook

    mod.set_axon_ntff_profile_hook = set_axon_ntff_profile_hook
    mod.get_axon_ntff_profile_hook = get_axon_ntff_profile_hook
    sys.modules["antenv.axon_hooks"] = mod
    antenv.axon_hooks = mod


try:
    _ensure_axon_hooks()
except Exception:
    pass

F32 = mybir.dt.float32
BF16 = mybir.dt.bfloat16
FP8 = mybir.dt.float8e3

B, N, H, D, WF = 2, 1024, 64, 32, 4
NCORES = 8
ROWS_PER_CORE = (B * N) // NCORES  # 256
RB = 16  # rows per j-block
N_JBLK = ROWS_PER_CORE // RB  # 16
N_KCHUNK = N // 128  # 8 s-chunks of 128
FH = WF * H  # 256
Q = ROWS_PER_CORE // 4  # 64 rows per tail quarter
LN_EPS = 1e-6
N_WARM = 5
NEWTON_ITERS = 1
# s-chunks 0..K8-1 stream as fp8 e3m4 (values pre-scaled by KS, with 1/KS
# folded into those chunks' x tiles); chunks K8..7 stream as bf16.
# K8=4 @ KS=2.5 measures 1.10e-2 fro vs the 2e-2 gate (HW-verified to
# match the numpy quantization model to ~1e-4; ~1.6e-2 even if the PE
# flushed fp8 denormals, which it doesn't).
K8 = 4
K16 = N_KCHUNK - K8
KS = 2.5

# consts blob column layout (f32 words)
BC_WB = 0  # [0:64, 0:1024]  wb2: W[d,c] tiled per r, spanning a j-block pair
BC_B2 = 1024  # [0:64, ...:+64] b2 broadcast over rows
BC_CB = 1088  # [0:64, ...:+1] conv_bias
BC_W = 1096
# bf16 blob (MLP weights run in bf16: fp32 matmuls cost 2 HW passes)
BH_W1 = 0  # [0:65, 0:256]   [ln_scale*W1 ; ln_bias@W1+b1]
BH_W2 = 256  # [0:128, 256:384] W2 as [p, fh*64+h]
BH_W = 384

_NC_CACHE = None
LAST_EXEC_NS = None


def _build_nc(has_cb=True, split_waits=True):
    nc = bass.Bass(target_bir_lowering=False)

    kb8 = nc.dram_tensor("kb8", [N_JBLK, 128, K8, RB, D], FP8, kind="ExternalInput")
    kbh = nc.dram_tensor("kbh", [N_JBLK, 128, K16, RB, D], BF16, kind="ExternalInput")
    xcp = nc.dram_tensor("xcp", [128, N_KCHUNK, H], BF16, kind="ExternalInput")
    blob = nc.dram_tensor("blob", [128, BC_W], F32, kind="ExternalInput")
    blob16 = nc.dram_tensor("blob16", [128, BH_W], BF16, kind="ExternalInput")
    out = nc.dram_tensor("out", [ROWS_PER_CORE, H], F32, kind="ExternalOutput")

    with tile.TileContext(nc) as tc:
        with (
            tc.tile_pool(name="consts", bufs=1) as consts,
            tc.tile_pool(name="kb8p", bufs=N_JBLK) as kb8_pool,
            tc.tile_pool(name="kbhp", bufs=N_JBLK) as kbh_pool,
            tc.tile_pool(name="mwp", bufs=3) as mw_pool,
            tc.tile_pool(name="tmpp", bufs=2) as tmp_pool,
            tc.tile_pool(name="work", bufs=2) as work,
            tc.tile_pool(name="pmain", bufs=4, space="PSUM") as pmain,
            tc.tile_pool(name="pstatp", bufs=1, space="PSUM") as pstatp,
            tc.tile_pool(name="ptail", bufs=3, space="PSUM") as ptail,
        ):
            # ---- x + consts blobs ride the Scalar engine's DMA queue so
            # the sync queue is kb-only and the stream starts immediately ----
            xc_sb = consts.tile([128, N_KCHUNK, H], BF16)
            nc.scalar.dma_start(out=xc_sb, in_=xcp[:, :, :])
            blob_sb = consts.tile([128, BC_W], F32)
            nc.scalar.dma_start(out=blob_sb, in_=blob[:, :])
            blob16_sb = consts.tile([128, BH_W], BF16)
            nc.scalar.dma_start(out=blob16_sb, in_=blob16[:, :])

            wb_sb = blob_sb[0:H, BC_WB : BC_WB + 2 * RB * D]
            b2_sb = blob_sb[0:Q, BC_B2 : BC_B2 + H]
            cb_sb = blob_sb[0:H, BC_CB : BC_CB + 1]
            w1_sb = blob16_sb[0 : H + 1, BH_W1 : BH_W1 + FH]
            w2_sb = blob16_sb[:, BH_W2 : BH_W2 + 2 * H]

            # ---- the kernel_basis stream: all ~12 MB prefetched; the last
            # j-block arrives in 3 pieces so PE can chase the stream (finer
            # splits trickle: sub-2KB descriptors run well below bus rate) ----
            kb_tiles = {}
            for j0 in range(N_JBLK):
                t8 = kb8_pool.tile([128, K8, RB, D], FP8, name=f"kb8_t{j0}", tag="kb8_t")
                t16 = kbh_pool.tile([128, K16, RB, D], BF16, name=f"kbh_t{j0}", tag="kbh_t")
                kb_tiles[j0] = (t8, t16)
                nc.sync.dma_start(out=t8, in_=kb8[j0, :, :, :, :])
                if j0 == N_JBLK - 1:
                    half = K16 // 2
                    nc.sync.dma_start(
                        out=t16[:, 0:half, :, :], in_=kbh[j0, :, 0:half, :, :]
                    )
                    nc.sync.dma_start(
                        out=t16[:, half:, :, :], in_=kbh[j0, :, half:, :, :]
                    )
                else:
                    nc.sync.dma_start(out=t16, in_=kbh[j0, :, :, :, :])

            # ---- small on-chip constants (GpSimd, idle otherwise) ----
            wtile = consts.tile([128, RB * D], BF16)
            nc.vector.memset(wtile, 0.25)
            ones64 = consts.tile([H, 1], BF16)
            nc.gpsimd.memset(ones64, 1.0)
            ones1 = consts.tile([1, H], BF16)
            nc.gpsimd.memset(ones1, 1.0)
            z_sb = consts.tile([H + 1, Q], BF16)
            nc.gpsimd.memset(z_sb[H : H + 1, :], 1.0)
            rp = consts.tile([1, 2 * Q], BF16)
            stv = consts.tile([H, 2, ROWS_PER_CORE], BF16)  # [a+cb ; (a+cb)^2]

            # ---- PE warm-up on a memset tile: the ~3us HAM ramp to 2.4 GHz
            # runs before the first kb tile lands, so j0 starts at full speed ----
            ps_warm = pmain.tile([H, RB * D], F32, name="ps", tag="ps")
            for w in range(N_WARM):
                nc.tensor.matmul(
                    ps_warm,
                    lhsT=wtile[:, 0:H],
                    rhs=wtile,
                    start=True,
                    stop=True,
                )

            # persistent LN-stats accumulator: [1, (sum, sum-of-squares), j, r]
            pstat = pstatp.tile([1, 2, N_JBLK, RB], F32)

            # ---- tail pieces per row-quarter, staggered through the j-loop
            # so every engine-queue entry's inputs are long-ready. Quarters
            # 0-2 run their scalar chains on the idle GpSimd engine to keep
            # the DVE queue clear for the stream; the post-stream quarter 3
            # uses DVE (empty by then, lower op latency). ----
            state = {}

            def t_qt(q):
                stats_mm4(q)
                eng = nc.vector
                sl4 = slice(4 * q, 4 * (q + 1))
                sa_v, ssq_v = pstat[:, 0, sl4, :], pstat[:, 1, sl4, :]
                qt = work.tile([1, Q], F32, name=f"qt{q}", tag="qt")
                eng.tensor_scalar(
                    out=qt, in0=ssq_v, scalar1=1.0 / H, scalar2=LN_EPS,
                    op0=mybir.AluOpType.mult, op1=mybir.AluOpType.add,
                )
                mu = work.tile([1, Q], F32, name=f"mu{q}", tag="mu")
                eng.tensor_scalar(
                    out=mu, in0=sa_v, scalar1=-1.0 / H, scalar2=None,
                    op0=mybir.AluOpType.mult,
                )
                t3 = work.tile([1, Q], F32, name=f"t3_{q}", tag="t3")
                eng.tensor_mul(t3, mu, mu)
                eng.tensor_sub(qt, qt, t3)
                state[("qt", q)] = qt
                state[("mu", q)] = mu

            def t_newton(q):
                eng = nc.vector
                qt = state[("qt", q)]
                mu = state[("mu", q)]
                # rsqrt without ScalarE (its LUT stays pinned on gelu):
                # quake seed via int<->float value casts + Newton steps.
                yi = work.tile([1, Q], mybir.dt.int32, name=f"yi{q}", tag="yi")
                eng.tensor_scalar(
                    out=yi, in0=qt.bitcast(mybir.dt.int32), scalar1=-0.5,
                    scalar2=float(0x5F3759DF),
                    op0=mybir.AluOpType.mult, op1=mybir.AluOpType.add,
                )
                y = yi.bitcast(F32)
                t1 = work.tile([1, Q], F32, name=f"t1_{q}", tag="t1")
                for it in range(NEWTON_ITERS):
                    eng.tensor_mul(t1, y, y)
                    eng.tensor_mul(t1, t1, qt)
                    eng.tensor_scalar(
                        out=t1, in0=t1, scalar1=-0.5, scalar2=1.5,
                        op0=mybir.AluOpType.mult, op1=mybir.AluOpType.add,
                    )
                    if it == NEWTON_ITERS - 1:
                        eng.tensor_mul(rp[:, 0:Q], y, t1)
                    else:
                        eng.tensor_mul(y, y, t1)
                eng.tensor_mul(rp[:, Q : 2 * Q], rp[:, 0:Q], mu)

            def t_bc(q):
                ps_bc = ptail.tile([H, 2 * Q], F32, name=f"ps_bc{q}", tag="ps_bc", bufs=1)
                nc.tensor.matmul(ps_bc, lhsT=ones1, rhs=rp, start=True, stop=True)
                nc.vector.tensor_mul(
                    z_sb[0:H, :], stv[:, 0, Q * q : Q * (q + 1)], ps_bc[:, 0:Q]
                )
                nc.vector.tensor_add(z_sb[0:H, :], z_sb[0:H, :], ps_bc[:, Q : 2 * Q])

            def t_mlp_a(q):
                ph = ptail.tile([128, 2, Q], F32, name=f"ph{q}", tag="ph", bufs=1)
                for fh in range(2):
                    nc.tensor.matmul(
                        ph[:, fh, :],
                        lhsT=w1_sb[:, 128 * fh : 128 * (fh + 1)],
                        rhs=z_sb,
                        start=True,
                        stop=True,
                    )
                hT = work.tile([128, 2, Q], BF16, name=f"hT{q}", tag="hT")
                for fh in range(2):
                    nc.scalar.activation(
                        out=hT[:, fh, :],
                        in_=ph[:, fh, :],
                        func=mybir.ActivationFunctionType.Gelu_apprx_tanh,
                        bias=0.0,
                        scale=1.0,
                    )
                state[("hT", q)] = hT

            def t_mlp_b(q):
                hT = state[("hT", q)]
                po = ptail.tile([Q, H], F32, name=f"po{q}", tag="po", bufs=1)
                for fh in range(2):
                    nc.tensor.matmul(
                        po,
                        lhsT=hT[:, fh, :],
                        rhs=w2_sb[:, H * fh : H * (fh + 1)],
                        start=(fh == 0),
                        stop=(fh == 1),
                    )
                o_sb = work.tile([Q, H], F32, name=f"o_sb{q}", tag="o_sb")
                nc.vector.tensor_add(o_sb, po, b2_sb)
                eng = nc.scalar if q == 3 else nc.sync
                eng.dma_start(out=out[Q * q : Q * (q + 1), :], in_=o_sb)

            sched = {}
            for q in range(3):
                sched.setdefault(4 * q + 4, []).append(lambda q=q: t_qt(q))
                sched.setdefault(4 * q + 5, []).append(lambda q=q: t_newton(q))
                sched.setdefault(4 * q + 6, []).append(lambda q=q: t_bc(q))
                if 4 * q + 7 < N_JBLK - 1:
                    sched.setdefault(4 * q + 7, []).append(lambda q=q: t_mlp_a(q))
                if 4 * q + 8 < N_JBLK:
                    # runs ahead of the same slot's qt so the out DMA fires asap
                    sched.setdefault(4 * q + 8, []).insert(0, lambda q=q: t_mlp_b(q))

            def stats_mm4(q):
                # channel sums for the quarter's 4 j-blocks; runs right when
                # PE reaches it (the last producer finished a slot earlier)
                nc.tensor.matmul(
                    pstat[:, :, 4 * q : 4 * q + 4, :],
                    lhsT=ones64,
                    rhs=stv[:, :, Q * q : Q * (q + 1)],
                    start=True,
                    stop=True,
                )

            # ---- main contraction ----
            for j in range(N_JBLK):
                kb8_t, kbh_t = kb_tiles.pop(j)
                ps = pmain.tile([H, RB * D], F32, name="ps", tag="ps")
                for k in range(N_KCHUNK):
                    rhs = kb8_t[:, k, :, :] if k < K8 else kbh_t[:, k - K8, :, :]
                    nc.tensor.matmul(
                        ps, lhsT=xc_sb[:, k, :], rhs=rhs,
                        start=(k == 0), stop=(k == N_KCHUNK - 1),
                    )
                mw = mw_pool.tile([H, RB, D], BF16)
                nc.vector.tensor_mul(
                    mw.rearrange("p a b -> p (a b)"), ps, wb_sb[:, 0 : RB * D]
                )
                st0 = stv[:, 0, RB * j : RB * (j + 1)]
                if has_cb:
                    tmp = tmp_pool.tile([H, RB], F32)
                    nc.vector.tensor_reduce(
                        out=tmp,
                        in_=mw,
                        axis=mybir.AxisListType.X,
                        op=mybir.AluOpType.add,
                    )
                    nc.vector.tensor_scalar(
                        out=st0, in0=tmp, scalar1=cb_sb, scalar2=None,
                        op0=mybir.AluOpType.add,
                    )
                else:
                    # bf16 out: 2x DVE rate; LN-stats precision is budgeted
                    with nc.allow_low_precision("LN stats tolerate bf16"):
                        nc.vector.tensor_reduce(
                            out=st0,
                            in_=mw,
                            axis=mybir.AxisListType.X,
                            op=mybir.AluOpType.add,
                        )
                if j == N_JBLK - 1:
                    # post-stream critical path: stay on DVE, skip two hops
                    nc.vector.tensor_mul(stv[:, 1, RB * j : RB * (j + 1)], st0, st0)
                else:
                    nc.scalar.activation(
                        out=stv[:, 1, RB * j : RB * (j + 1)],
                        in_=st0,
                        func=mybir.ActivationFunctionType.Square,
                        bias=0.0,
                        scale=1.0,
                    )
                for fn in sched.get(j, ()):
                    fn()

            # last quarter after the stream; q2's MLP rides under q3's
            # DVE chain
            t_qt(3)
            t_mlp_a(2)
            t_newton(3)
            t_mlp_b(2)
            t_bc(3)
            t_mlp_a(3)
            t_mlp_b(3)

    if split_waits:
        _split_matmul_waits(nc)
    return nc


def _split_matmul_waits(nc):
    """This walrus build rejects engine instructions carrying more than one
    semaphore wait ("Too many sync wait commands"). Peel all but the last
    wait off onto same-engine NoOps inserted immediately before the
    instruction — NoOps execute in queue order on the same sequencer, so the
    wait semantics are unchanged."""
    f = nc.m.functions[0]
    nop_id = 0
    for blk in f.blocks:
        insts = list(blk.instructions)
        out = []
        changed = False
        for inst in insts:
            si = inst.sync_info
            if (
                si is not None
                and si.on_wait is not None
                and len(si.on_wait) > 1
                and getattr(inst, "engine", None) is not None
            ):
                waits = list(si.on_wait)
                for w in waits[:-1]:
                    nop = mybir.InstNoOp(
                        name=f"I-mmwait-{nop_id}",
                        engine=inst.engine,
                        ins=[],
                        outs=[],
                        sync_info=mybir.SyncInfo(on_wait=[w], on_update=[]),
                    )
                    nop_id += 1
                    out.append(nop)
                inst.sync_info = mybir.SyncInfo(
                    on_wait=[waits[-1]], on_update=list(si.on_update or [])
                )
                changed = True
            out.append(inst)
        if changed:
            blk.instructions = out
    return nc


def _get_nc(has_cb):
    global _NC_CACHE
    if _NC_CACHE is None or _NC_CACHE[0] != has_cb:
        _NC_CACHE = (has_cb, _build_nc(has_cb=has_cb))
    return _NC_CACHE[1]


def _prep_blob(kernel_W, conv_bias, ln_scale, ln_bias, W1, b1, W2, b2):
    import ml_dtypes

    blob = np.zeros((128, BC_W), np.float32)
    # wb2[c, (j2, r, d)] = W[d, c], spanning a pair of j-blocks
    blob[0:H, BC_WB : BC_WB + 2 * RB * D] = np.tile(kernel_W.T, (1, 2 * RB))
    blob[0:Q, BC_B2 : BC_B2 + H] = np.broadcast_to(b2, (Q, H))
    blob[0:H, BC_CB] = conv_bias

    blob16 = np.zeros((128, BH_W), ml_dtypes.bfloat16)
    # ln_scale folded into W1 rows; ln_bias@W1 + b1 as the K=65 bias row
    blob16[0:H, BH_W1 : BH_W1 + FH] = W1 * ln_scale[:, None]
    blob16[H, BH_W1 : BH_W1 + FH] = ln_bias @ W1 + b1
    blob16[:, BH_W2 : BH_W2 + 2 * H] = (
        W2.reshape(2, 128, H).transpose(1, 0, 2).reshape(128, 2 * H)
    )
    return blob, blob16


def _prep_x(xb):
    # (N, H) -> (128, k, H) bf16, with s = 128*k + p; the fp8 chunks'
    # kernel_basis is pre-scaled by KS, undone here
    import ml_dtypes

    xs = xb.reshape(N_KCHUNK, 128, H).copy()
    xs[:K8] *= 1.0 / KS
    xh = xs.astype(ml_dtypes.bfloat16)
    return np.ascontiguousarray(xh.transpose(1, 0, 2))


def _prep_kb_shard(shard):
    # shard (256, 1024, 32) -> (j, p, k, r^, d); s-chunks < K8 as e3m4*KS,
    # the rest bf16
    import ml_dtypes

    t = shard.reshape(N_JBLK, RB, N_KCHUNK, 128, D).transpose(0, 3, 2, 1, 4)
    lo = np.ascontiguousarray(t[:, :, :K8] * KS).astype(ml_dtypes.float8_e3m4)
    hi = np.ascontiguousarray(t[:, :, K8:]).astype(ml_dtypes.bfloat16)
    return lo, hi


def kernel(
    x,
    kernel_basis,
    kernel_W,
    conv_bias,
    ln_scale,
    ln_bias,
    W1,
    b1,
    W2,
    b2,
):
    global LAST_EXEC_NS
    x = np.ascontiguousarray(np.asarray(x, np.float32))
    kb = np.ascontiguousarray(np.asarray(kernel_basis, np.float32))
    blob, blob16 = _prep_blob(
        np.asarray(kernel_W, np.float32),
        np.asarray(conv_bias, np.float32),
        np.asarray(ln_scale, np.float32),
        np.asarray(ln_bias, np.float32),
        np.asarray(W1, np.float32),
        np.asarray(b1, np.float32),
        np.asarray(W2, np.float32),
        np.asarray(b2, np.float32),
    )
    xps = [_prep_x(x[b]) for b in range(B)]

    kbf = kb.reshape(B * N, N, D)
    in_maps = []
    for c in range(NCORES):
        lo, hi = _prep_kb_shard(kbf[c * ROWS_PER_CORE : (c + 1) * ROWS_PER_CORE])
        in_maps.append(
            dict(kb8=lo, kbh=hi, xcp=xps[c // (NCORES // B)], blob=blob, blob16=blob16)
        )

    nc = _get_nc(bool(np.any(np.asarray(conv_bias))))
    trace = bool(os.environ.get("KERNEL_BASS_TRACE"))
    res = run_bass_kernel_spmd(nc, in_maps, core_ids=list(range(NCORES)), trace=trace)
    LAST_EXEC_NS = res.exec_time_ns

    outs = np.concatenate([res.results[c]["out"] for c in range(NCORES)], axis=0)
    return outs.reshape(B, N, H)


# revision 69
# speedup vs baseline: 1.2025x; 1.2025x over previous
"""Trainium2 Bass kernel for nn_ConvBlock (SepGconv + LayerNorm + GELU MLP).

Computes, for full inputs:
    a   = einsum('bsc,brsd,dc->brc', x, kernel_basis, kernel_W) + conv_bias
    a   = LayerNorm(a) * ln_scale + ln_bias          (over channels, eps=1e-6)
    out = gelu_tanh(a @ W1 + b1) @ W2 + b2

Shapes: B=2, N=1024 (R=S=N), H=64, D=32, WF=4.

Sharding: the (B*R)=2048 output rows split into 8 contiguous shards of 256
rows, one per NeuronCore. Each core reads its kernel_basis shard once
(memory-bound), contracts over all S on-chip, and runs the LN/MLP tail
locally. x / weights are replicated.

Perf strategy (HBM-bound at ~360 GB/s per core; ~58.5 us vs the 135 us
session baseline):
- kernel_basis streams in mixed precision: s-chunks 0..K8-1 as fp8 e3m4
  (1 B/elem; values pre-scaled by KS, with 1/KS folded into those chunks'
  x tiles) and the rest bf16 (2 B/elem) -- ~12.6 MB/core. The gate is
  rel_err < 2e-2; this measures 1.10e-2, deterministic (fixed inputs,
  fixed quantization, f32 PSUM accumulation). All tiles are prefetched
  into SBUF up front (SBUF is ~26 MB) so the DMA queues never stall on
  buffer recycling; the last j-block arrives in 3 pieces so PE can chase
  it (finer splits trickle: sub-2KB descriptors run below bus rate).
- dma_start issue costs ~0.6 us of serial sequencer time each, so the kb
  stream is issued alone on the sync queue while x + all small constants
  ride in packed-blob DMAs issued from the (otherwise idle) Scalar
  engine's queue. MLP/LN weights are pre-folded and packed bf16 (fp32
  matmuls cost 2 HW passes, so the whole tail runs bf16).
- Each matmul is psum[c, (r,d)] += x[s,c]^T @ kb[s,(r,d)] with N=512
  (16 rows x 32 d), K=128 s-chunk; the d-reduction against kernel_W runs
  on DVE (multiply by W broadcast + free-axis reduce). PE warm-up runs on
  a memset tile so the ~3 us HAM ramp to 2.4 GHz hides under x's DMA
  latency (dropping it makes j0-j2 run at mid p-state: measured worse).
- LayerNorm stats are accumulated DURING the stream: per j-block the
  channel sums of a+cb and its square land in a persistent PSUM tile via
  a per-quarter ones-matmul (squares on the Scalar engine -- its gelu
  table also holds Square). Row quarters then only need a short
  rsqrt(quake seed + Newton)/scale/MLP chain, staggered through the
  j-loop so every in-order engine-queue entry's inputs are long-ready;
  ln_scale/ln_bias are folded into W1/b1 on the host. Only the last
  quarter's chain (~6 us) runs after the stream.
"""

import os

import numpy as np

import concourse.bass as bass
import concourse.tile as tile
from concourse import mybir
from concourse.bass_utils import run_bass_kernel_spmd


def _ensure_axon_hooks():
    """bass_utils imports antenv.axon_hooks when trace=True under axon; some
    images ship antenv without that module. Register a functional stand-in
    (driving NTFF capture via libaxon_pjrt.so) so tracing works, degrading
    to hook=None (no trace, run still works) if the .so is unavailable."""
    import sys
    import types

    try:
        import antenv.axon_hooks  # noqa: F401

        return
    except ImportError:
        pass
    try:
        import antenv
    except ImportError:
        antenv = types.ModuleType("antenv")
        sys.modules["antenv"] = antenv

    mod = types.ModuleType("antenv.axon_hooks")
    mod._hook = None

    def set_axon_ntff_profile_hook(h):
        mod._hook = h

    def get_axon_ntff_profile_hook():
        if mod._hook is None:
            try:
                from trn_agent_boot.trn_boot import _ntff_profile_via_ctypes

                so_path = "/opt/axon/libaxon_pjrt.so"
                if os.path.exists(so_path):
                    mod._hook = _ntff_profile_via_ctypes(so_path)
            except Exception:
                mod._hook = None
        return mod._hook

    mod.set_axon_ntff_profile_hook = set_axon_ntff_profile_hook
    mod.get_axon_ntff_profile_hook = get_axon_ntff_profile_hook
    sys.modules["antenv.axon_hooks"] = mod
    antenv.axon_hooks = mod


try:
    _ensure_axon_hooks()
except Exception:
    pass

F32 = mybir.dt.float32
BF16 = mybir.dt.bfloat16
FP8 = mybir.dt.float8e3

B, N, H, D, WF = 2, 1024, 64, 32, 4
NCORES = 8
ROWS_PER_CORE = (B * N) // NCORES  # 256
RB = 16  # rows per j-block
N_JBLK = ROWS_PER_CORE // RB  # 16
N_KCHUNK = N // 128  # 8 s-chunks of 128
FH = WF * H  # 256
Q = ROWS_PER_CORE // 4  # 64 rows per tail quarter
LN_EPS = 1e-6
N_WARM = 5
NEWTON_ITERS = 1
# s-chunks 0..K8-1 stream as fp8 e3m4 (values pre-scaled by KS, with 1/KS
# folded into those chunks' x tiles); chunks K8..7 stream as bf16.
# K8=4 @ KS=2.5 measures 1.10e-2 fro vs the 2e-2 gate (HW-verified to
# match the numpy quantization model to ~1e-4; ~1.6e-2 even if the PE
# flushed fp8 denormals, which it doesn't).
K8 = 4
K16 = N_KCHUNK - K8
KS = 2.5

# consts blob column layout (f32 words)
BC_WB = 0  # [0:64, 0:1024]  wb2: W[d,c] tiled per r, spanning a j-block pair
BC_B2 = 1024  # [0:64, ...:+64] b2 broadcast over rows
BC_CB = 1088  # [0:64, ...:+1] conv_bias
BC_W = 1096
# bf16 blob (MLP weights run in bf16: fp32 matmuls cost 2 HW passes)
BH_W1 = 0  # [0:65, 0:256]   [ln_scale*W1 ; ln_bias@W1+b1]
BH_W2 = 256  # [0:128, 256:384] W2 as [p, fh*64+h]
BH_W = 384

_NC_CACHE = None
LAST_EXEC_NS = None


def _build_nc(has_cb=True, split_waits=True):
    nc = bass.Bass(target_bir_lowering=False)

    kb8 = nc.dram_tensor("kb8", [N_JBLK, 128, K8, RB, D], FP8, kind="ExternalInput")
    kbh = nc.dram_tensor("kbh", [N_JBLK, 128, K16, RB, D], BF16, kind="ExternalInput")
    xcp = nc.dram_tensor("xcp", [128, N_KCHUNK, H], BF16, kind="ExternalInput")
    blob = nc.dram_tensor("blob", [128, BC_W], F32, kind="ExternalInput")
    blob16 = nc.dram_tensor("blob16", [128, BH_W], BF16, kind="ExternalInput")
    out = nc.dram_tensor("out", [ROWS_PER_CORE, H], F32, kind="ExternalOutput")

    with tile.TileContext(nc) as tc:
        with (
            tc.tile_pool(name="consts", bufs=1) as consts,
            tc.tile_pool(name="kb8p", bufs=N_JBLK) as kb8_pool,
            tc.tile_pool(name="kbhp", bufs=N_JBLK) as kbh_pool,
            tc.tile_pool(name="mwp", bufs=3) as mw_pool,
            tc.tile_pool(name="tmpp", bufs=2) as tmp_pool,
            tc.tile_pool(name="work", bufs=2) as work,
            tc.tile_pool(name="pmain", bufs=4, space="PSUM") as pmain,
            tc.tile_pool(name="pstatp", bufs=1, space="PSUM") as pstatp,
            tc.tile_pool(name="ptail", bufs=3, space="PSUM") as ptail,
        ):
            # ---- x + consts blobs ride the Scalar engine's DMA queue so
            # the sync queue is kb-only and the stream starts immediately ----
            xc_sb = consts.tile([128, N_KCHUNK, H], BF16)
            nc.scalar.dma_start(out=xc_sb, in_=xcp[:, :, :])
            blob_sb = consts.tile([128, BC_W], F32)
            nc.scalar.dma_start(out=blob_sb, in_=blob[:, :])
            blob16_sb = consts.tile([128, BH_W], BF16)
            nc.scalar.dma_start(out=blob16_sb, in_=blob16[:, :])

            wb_sb = blob_sb[0:H, BC_WB : BC_WB + 2 * RB * D]
            b2_sb = blob_sb[0:Q, BC_B2 : BC_B2 + H]
            cb_sb = blob_sb[0:H, BC_CB : BC_CB + 1]
            w1_sb = blob16_sb[0 : H + 1, BH_W1 : BH_W1 + FH]
            w2_sb = blob16_sb[:, BH_W2 : BH_W2 + 2 * H]

            # ---- the kernel_basis stream: all ~12 MB prefetched; the last
            # j-block arrives in 3 pieces so PE can chase the stream (finer
            # splits trickle: sub-2KB descriptors run well below bus rate) ----
            kb_tiles = {}
            for j0 in range(N_JBLK):
                t8 = kb8_pool.tile([128, K8, RB, D], FP8, name=f"kb8_t{j0}", tag="kb8_t")
                t16 = kbh_pool.tile([128, K16, RB, D], BF16, name=f"kbh_t{j0}", tag="kbh_t")
                kb_tiles[j0] = (t8, t16)
                nc.sync.dma_start(out=t8, in_=kb8[j0, :, :, :, :])
                if j0 == N_JBLK - 1:
                    half = K16 // 2
                    nc.sync.dma_start(
                        out=t16[:, 0:half, :, :], in_=kbh[j0, :, 0:half, :, :]
                    )
                    nc.sync.dma_start(
                        out=t16[:, half:, :, :], in_=kbh[j0, :, half:, :, :]
                    )
                else:
                    nc.sync.dma_start(out=t16, in_=kbh[j0, :, :, :, :])

            # ---- small on-chip constants (GpSimd, idle otherwise) ----
            wtile = consts.tile([128, RB * D], BF16)
            nc.vector.memset(wtile, 0.25)
            ones64 = consts.tile([H, 1], F32)
            nc.gpsimd.memset(ones64, 1.0)
            ones1 = consts.tile([1, H], BF16)
            nc.gpsimd.memset(ones1, 1.0)
            z_sb = consts.tile([H + 1, Q], BF16)
            nc.gpsimd.memset(z_sb[H : H + 1, :], 1.0)
            rp = consts.tile([1, 2 * Q], BF16)
            stv = consts.tile([H, 2, ROWS_PER_CORE], F32)  # [a+cb ; (a+cb)^2]

            # ---- PE warm-up on a memset tile: the ~3us HAM ramp to 2.4 GHz
            # runs before the first kb tile lands, so j0 starts at full speed ----
            ps_warm = pmain.tile([H, RB * D], F32, name="ps", tag="ps")
            for w in range(N_WARM):
                nc.tensor.matmul(
                    ps_warm,
                    lhsT=wtile[:, 0:H],
                    rhs=wtile,
                    start=True,
                    stop=True,
                )

            # persistent LN-stats accumulator: [1, (sum, sum-of-squares), j, r]
            pstat = pstatp.tile([1, 2, N_JBLK, RB], F32)

            # ---- tail pieces per row-quarter, staggered through the j-loop
            # so every engine-queue entry's inputs are long-ready. Quarters
            # 0-2 run their scalar chains on the idle GpSimd engine to keep
            # the DVE queue clear for the stream; the post-stream quarter 3
            # uses DVE (empty by then, lower op latency). ----
            state = {}

            def t_qt(q):
                stats_mm4(q)
                eng = nc.vector
                sl4 = slice(4 * q, 4 * (q + 1))
                sa_v, ssq_v = pstat[:, 0, sl4, :], pstat[:, 1, sl4, :]
                qt = work.tile([1, Q], F32, name=f"qt{q}", tag="qt")
                eng.tensor_scalar(
                    out=qt, in0=ssq_v, scalar1=1.0 / H, scalar2=LN_EPS,
                    op0=mybir.AluOpType.mult, op1=mybir.AluOpType.add,
                )
                mu = work.tile([1, Q], F32, name=f"mu{q}", tag="mu")
                eng.tensor_scalar(
                    out=mu, in0=sa_v, scalar1=-1.0 / H, scalar2=None,
                    op0=mybir.AluOpType.mult,
                )
                t3 = work.tile([1, Q], F32, name=f"t3_{q}", tag="t3")
                eng.tensor_mul(t3, mu, mu)
                eng.tensor_sub(qt, qt, t3)
                state[("qt", q)] = qt
                state[("mu", q)] = mu

            def t_newton(q):
                eng = nc.vector
                qt = state[("qt", q)]
                mu = state[("mu", q)]
                # rsqrt without ScalarE (its LUT stays pinned on gelu):
                # quake seed via int<->float value casts + Newton steps.
                yi = work.tile([1, Q], mybir.dt.int32, name=f"yi{q}", tag="yi")
                eng.tensor_scalar(
                    out=yi, in0=qt.bitcast(mybir.dt.int32), scalar1=-0.5,
                    scalar2=float(0x5F3759DF),
                    op0=mybir.AluOpType.mult, op1=mybir.AluOpType.add,
                )
                y = yi.bitcast(F32)
                t1 = work.tile([1, Q], F32, name=f"t1_{q}", tag="t1")
                for it in range(NEWTON_ITERS):
                    eng.tensor_mul(t1, y, y)
                    eng.tensor_mul(t1, t1, qt)
                    eng.tensor_scalar(
                        out=t1, in0=t1, scalar1=-0.5, scalar2=1.5,
                        op0=mybir.AluOpType.mult, op1=mybir.AluOpType.add,
                    )
                    if it == NEWTON_ITERS - 1:
                        eng.tensor_mul(rp[:, 0:Q], y, t1)
                    else:
                        eng.tensor_mul(y, y, t1)
                eng.tensor_mul(rp[:, Q : 2 * Q], rp[:, 0:Q], mu)

            def t_bc(q):
                ps_bc = ptail.tile([H, 2 * Q], F32, name=f"ps_bc{q}", tag="ps_bc", bufs=1)
                nc.tensor.matmul(ps_bc, lhsT=ones1, rhs=rp, start=True, stop=True)
                nc.vector.tensor_mul(
                    z_sb[0:H, :], stv[:, 0, Q * q : Q * (q + 1)], ps_bc[:, 0:Q]
                )
                nc.vector.tensor_add(z_sb[0:H, :], z_sb[0:H, :], ps_bc[:, Q : 2 * Q])

            def t_mlp_a(q):
                ph = ptail.tile([128, 2, Q], F32, name=f"ph{q}", tag="ph", bufs=1)
                for fh in range(2):
                    nc.tensor.matmul(
                        ph[:, fh, :],
                        lhsT=w1_sb[:, 128 * fh : 128 * (fh + 1)],
                        rhs=z_sb,
                        start=True,
                        stop=True,
                    )
                hT = work.tile([128, 2, Q], BF16, name=f"hT{q}", tag="hT")
                for fh in range(2):
                    nc.scalar.activation(
                        out=hT[:, fh, :],
                        in_=ph[:, fh, :],
                        func=mybir.ActivationFunctionType.Gelu_apprx_tanh,
                        bias=0.0,
                        scale=1.0,
                    )
                state[("hT", q)] = hT

            def t_mlp_b(q):
                hT = state[("hT", q)]
                po = ptail.tile([Q, H], F32, name=f"po{q}", tag="po", bufs=1)
                for fh in range(2):
                    nc.tensor.matmul(
                        po,
                        lhsT=hT[:, fh, :],
                        rhs=w2_sb[:, H * fh : H * (fh + 1)],
                        start=(fh == 0),
                        stop=(fh == 1),
                    )
                o_sb = work.tile([Q, H], F32, name=f"o_sb{q}", tag="o_sb")
                nc.vector.tensor_add(o_sb, po, b2_sb)
                eng = nc.scalar if q == 3 else nc.sync
                eng.dma_start(out=out[Q * q : Q * (q + 1), :], in_=o_sb)

            sched = {}
            for q in range(3):
                sched.setdefault(4 * q + 4, []).append(lambda q=q: t_qt(q))
                sched.setdefault(4 * q + 5, []).append(lambda q=q: t_newton(q))
                sched.setdefault(4 * q + 6, []).append(lambda q=q: t_bc(q))
                if 4 * q + 7 < N_JBLK - 1:
                    sched.setdefault(4 * q + 7, []).append(lambda q=q: t_mlp_a(q))
                if 4 * q + 8 < N_JBLK:
                    # runs ahead of the same slot's qt so the out DMA fires asap
                    sched.setdefault(4 * q + 8, []).insert(0, lambda q=q: t_mlp_b(q))

            def stats_mm4(q):
                # channel sums for the quarter's 4 j-blocks; runs right when
                # PE reaches it (the last producer finished a slot earlier)
                nc.tensor.matmul(
                    pstat[:, :, 4 * q : 4 * q + 4, :],
                    lhsT=ones64,
                    rhs=stv[:, :, Q * q : Q * (q + 1)],
                    start=True,
                    stop=True,
                )

            # ---- main contraction ----
            for j in range(N_JBLK):
                kb8_t, kbh_t = kb_tiles.pop(j)
                ps = pmain.tile([H, RB * D], F32, name="ps", tag="ps")
                for k in range(N_KCHUNK):
                    rhs = kb8_t[:, k, :, :] if k < K8 else kbh_t[:, k - K8, :, :]
                    nc.tensor.matmul(
                        ps, lhsT=xc_sb[:, k, :], rhs=rhs,
                        start=(k == 0), stop=(k == N_KCHUNK - 1),
                    )
                mw = mw_pool.tile([H, RB, D], BF16)
                nc.vector.tensor_mul(
                    mw.rearrange("p a b -> p (a b)"), ps, wb_sb[:, 0 : RB * D]
                )
                st0 = stv[:, 0, RB * j : RB * (j + 1)]
                if has_cb:
                    tmp = tmp_pool.tile([H, RB], F32)
                    nc.vector.tensor_reduce(
                        out=tmp,
                        in_=mw,
                        axis=mybir.AxisListType.X,
                        op=mybir.AluOpType.add,
                    )
                    nc.vector.tensor_scalar(
                        out=st0, in0=tmp, scalar1=cb_sb, scalar2=None,
                        op0=mybir.AluOpType.add,
                    )
                else:
                    nc.vector.tensor_reduce(
                        out=st0,
                        in_=mw,
                        axis=mybir.AxisListType.X,
                        op=mybir.AluOpType.add,
                    )
                if j == N_JBLK - 1:
                    # post-stream critical path: stay on DVE, skip two hops
                    nc.vector.tensor_mul(stv[:, 1, RB * j : RB * (j + 1)], st0, st0)
                else:
                    nc.scalar.activation(
                        out=stv[:, 1, RB * j : RB * (j + 1)],
                        in_=st0,
                        func=mybir.ActivationFunctionType.Square,
                        bias=0.0,
                        scale=1.0,
                    )
                for fn in sched.get(j, ()):
                    fn()

            # last quarter after the stream; q2's MLP rides under q3's
            # DVE chain
            t_qt(3)
            t_mlp_a(2)
            t_newton(3)
            t_mlp_b(2)
            t_bc(3)
            t_mlp_a(3)
            t_mlp_b(3)

    if split_waits:
        _split_matmul_waits(nc)
    return nc


def _split_matmul_waits(nc):
    """This walrus build rejects engine instructions carrying more than one
    semaphore wait ("Too many sync wait commands"). Peel all but the last
    wait off onto same-engine NoOps inserted immediately before the
    instruction — NoOps execute in queue order on the same sequencer, so the
    wait semantics are unchanged."""
    f = nc.m.functions[0]
    nop_id = 0
    for blk in f.blocks:
        insts = list(blk.instructions)
        out = []
        changed = False
        for inst in insts:
            si = inst.sync_info
            if (
                si is not None
                and si.on_wait is not None
                and len(si.on_wait) > 1
                and getattr(inst, "engine", None) is not None
            ):
                waits = list(si.on_wait)
                for w in waits[:-1]:
                    nop = mybir.InstNoOp(
                        name=f"I-mmwait-{nop_id}",
                        engine=inst.engine,
                        ins=[],
                        outs=[],
                        sync_info=mybir.SyncInfo(on_wait=[w], on_update=[]),
                    )
                    nop_id += 1
                    out.append(nop)
                inst.sync_info = mybir.SyncInfo(
                    on_wait=[waits[-1]], on_update=list(si.on_update or [])
                )
                changed = True
            out.append(inst)
        if changed:
            blk.instructions = out
    return nc


def _get_nc(has_cb):
    global _NC_CACHE
    if _NC_CACHE is None or _NC_CACHE[0] != has_cb:
        _NC_CACHE = (has_cb, _build_nc(has_cb=has_cb))
    return _NC_CACHE[1]


def _prep_blob(kernel_W, conv_bias, ln_scale, ln_bias, W1, b1, W2, b2):
    import ml_dtypes

    blob = np.zeros((128, BC_W), np.float32)
    # wb2[c, (j2, r, d)] = W[d, c], spanning a pair of j-blocks
    blob[0:H, BC_WB : BC_WB + 2 * RB * D] = np.tile(kernel_W.T, (1, 2 * RB))
    blob[0:Q, BC_B2 : BC_B2 + H] = np.broadcast_to(b2, (Q, H))
    blob[0:H, BC_CB] = conv_bias

    blob16 = np.zeros((128, BH_W), ml_dtypes.bfloat16)
    # ln_scale folded into W1 rows; ln_bias@W1 + b1 as the K=65 bias row
    blob16[0:H, BH_W1 : BH_W1 + FH] = W1 * ln_scale[:, None]
    blob16[H, BH_W1 : BH_W1 + FH] = ln_bias @ W1 + b1
    blob16[:, BH_W2 : BH_W2 + 2 * H] = (
        W2.reshape(2, 128, H).transpose(1, 0, 2).reshape(128, 2 * H)
    )
    return blob, blob16


def _prep_x(xb):
    # (N, H) -> (128, k, H) bf16, with s = 128*k + p; the fp8 chunks'
    # kernel_basis is pre-scaled by KS, undone here
    import ml_dtypes

    xs = xb.reshape(N_KCHUNK, 128, H).copy()
    xs[:K8] *= 1.0 / KS
    xh = xs.astype(ml_dtypes.bfloat16)
    return np.ascontiguousarray(xh.transpose(1, 0, 2))


def _prep_kb_shard(shard):
    # shard (256, 1024, 32) -> (j, p, k, r^, d); s-chunks < K8 as e3m4*KS,
    # the rest bf16
    import ml_dtypes

    t = shard.reshape(N_JBLK, RB, N_KCHUNK, 128, D).transpose(0, 3, 2, 1, 4)
    lo = np.ascontiguousarray(t[:, :, :K8] * KS).astype(ml_dtypes.float8_e3m4)
    hi = np.ascontiguousarray(t[:, :, K8:]).astype(ml_dtypes.bfloat16)
    return lo, hi


def kernel(
    x,
    kernel_basis,
    kernel_W,
    conv_bias,
    ln_scale,
    ln_bias,
    W1,
    b1,
    W2,
    b2,
):
    global LAST_EXEC_NS
    x = np.ascontiguousarray(np.asarray(x, np.float32))
    kb = np.ascontiguousarray(np.asarray(kernel_basis, np.float32))
    blob, blob16 = _prep_blob(
        np.asarray(kernel_W, np.float32),
        np.asarray(conv_bias, np.float32),
        np.asarray(ln_scale, np.float32),
        np.asarray(ln_bias, np.float32),
        np.asarray(W1, np.float32),
        np.asarray(b1, np.float32),
        np.asarray(W2, np.float32),
        np.asarray(b2, np.float32),
    )
    xps = [_prep_x(x[b]) for b in range(B)]

    kbf = kb.reshape(B * N, N, D)
    in_maps = []
    for c in range(NCORES):
        lo, hi = _prep_kb_shard(kbf[c * ROWS_PER_CORE : (c + 1) * ROWS_PER_CORE])
        in_maps.append(
            dict(kb8=lo, kbh=hi, xcp=xps[c // (NCORES // B)], blob=blob, blob16=blob16)
        )

    nc = _get_nc(bool(np.any(np.asarray(conv_bias))))
    trace = bool(os.environ.get("KERNEL_BASS_TRACE"))
    res = run_bass_kernel_spmd(nc, in_maps, core_ids=list(range(NCORES)), trace=trace)
    LAST_EXEC_NS = res.exec_time_ns

    outs = np.concatenate([res.results[c]["out"] for c in range(NCORES)], axis=0)
    return outs.reshape(B, N, H)
